# revision 21
# baseline (speedup 1.0000x reference)
"""DeepHisCoM Trainium2 kernel (nn_DeepHisCoM_7017976562218).

Math (reference):
    xr = x.reshape(B, P, V)
    z1 = einsum('bpv,pwv->bpw', xr, W1);  h = leaky(z1)          # per-pathway Linear V->W
    z2 = einsum('bpw,pw->bp', h, W2);     pval = leaky(z2)       # per-pathway Linear W->1
    BN(batch stats) -> global L2 normalize -> sigmoid(pn @ Wd + bd)

Device strategy (8 NeuronCores, batch-sharded 2048 rows/core):
    - x is cast to fp8(e4m3) and pre-transposed on the HOST into
      xt[v, (bt, p, b)] so the contraction dim V lands on partitions with
      zero device-side transposes (the old kernel burned half the PE on
      128x128 transposes, which also kept the HAM clock gate cold).
    - One 66-column fp8 matmul per pathway: lhsT = xt block (stationary,
      128 cols -> FWL fp8 weight load), rhs = [16*W1p^T | +256u | -256u]
      where u = 0.2 * W1p^T @ W2p.  leaky(z1) = 0.2*z1 + 0.8*relu(z1), so
      z2 = sum_w relu(z1)*0.8*W2 + (relu(q) - relu(-q)) with q = 0.2*sum_w z1*W2
      carried exactly by the +/-u columns through the uniform relu.
      Power-of-2 column scales (16, 256) keep fp8 operands in the normal
      range; the exact inverse rides in the bf16 w2e coefficients.
    - VectorE: fused prefix-sum of (w2e * relu(h)) in one custom DVE op,
      per-pathway sums recovered as boundary-column differences; final
      leaky via max(0.2*z, z).
    - BN stats + L2 norm + final linear + sigmoid on host (8 MiB, trivial).

fp8 is safe here: the global L2 norm + sigmoid make the logits tiny;
full-pipeline rel err vs the fp32 reference measured ~8e-6.
"""

import os
import sys

import numpy as np

for _p in ("/opt/trn_rl_repo",):
    if _p not in sys.path and os.path.isdir(_p):
        sys.path.insert(0, _p)

import ml_dtypes

import concourse.bacc as bacc
import concourse.bass as bass
import concourse.mybir as mybir
from concourse import dve_ops
from concourse.bass_utils import run_bass_kernel_spmd
from concourse.dve_spec import AluOp, Spec, Src0, Src1, Zero, relu, scan
from concourse.tile import TileContext


def _register_prefix_sum_op():
    """Fused DVE op: out[t] = running sum of in0[t] * relu(in1[t]).

    Per-pathway sums are recovered afterwards as differences of the
    segment-boundary columns of the prefix sum.
    """
    name = "STT_PREFIX_SUM_ANT"
    for op in dve_ops.OPS:
        if op.name == name:
            return op

    def ref(in0, in1, s0, s1, imm2):
        return np.cumsum(in0.astype(np.float32) * np.maximum(in1, 0), axis=-1)

    op = dve_ops.DveOp(
        name,
        Spec(body=scan(AluOp.ADD, Src0 * relu(Src1), init=Zero), reference=ref),
        subdim=False,
        uops_sha={"v3": "0179e875ac56dbc9", "v4": "d52b99774727e4db"},
    )
    dve_ops.OPS.append(op)
    dve_ops._SUB_OPCODE_FOR_NAME[name] = dve_ops._CUSTOM_DVE_ROW_BASE + len(dve_ops.OPS) - 1
    dve_ops.CUSTOM_DVE_SPECS[name] = op.spec
    return op


PREFIX_SUM_OP = _register_prefix_sum_op()

P, V, W = 128, 128, 64
B = 16384
N_CORES = 8
BSH = B // N_CORES          # 2048 batch rows per core
NBT = BSH // 128            # 16 batch tiles per core
BN_EPS = 1e-5
NCOL = W + 2                # 66: W1^T columns + (+u, -u)
S_W1 = 16.0                 # fp8 scale for the W1^T columns
S_U = 256.0                 # fp8 scale for the +/-u columns
F32 = mybir.dt.float32
BF16 = mybir.dt.bfloat16
F8 = mybir.dt.float8e4

# pathway groups per 64-pathway half: (start, size); size split across 2 PSUM banks
GROUPS = [(0, 14), (14, 14), (28, 14), (42, 14), (56, 8)]

_CACHE = {}
LAST_RESULTS = None


def _build_program():
    nc = bacc.Bacc()
    xt_in = nc.declare_dram_parameter("xt", [V, NBT * P * 128], F8, isOutput=False)
    wext_in = nc.declare_dram_parameter("wext", [V, P * NCOL], F8, isOutput=False)
    w2e_in = nc.declare_dram_parameter("w2e", [128, P * NCOL], BF16, isOutput=False)
    p_out = nc.declare_dram_parameter("ps", [BSH, P], F32, isOutput=True)

    with TileContext(nc) as tc:
        with (
            tc.tile_pool(name="singles", bufs=1) as singles,
            tc.tile_pool(name="xt", bufs=3) as xtp,
            tc.tile_pool(name="prod", bufs=2) as prodp,
            tc.tile_pool(name="psb", bufs=2) as psbp,
            tc.tile_pool(name="pf", bufs=2) as pfp,
            tc.tile_pool(name="hps", bufs=4, space="PSUM") as hpsp,
        ):
            # weight loads: wext rides the scalar ring ahead of that ring's x
            # chunks, in two pieces so the first matmul group starts sooner;
            # the tiny w2e row rides gpsimd and is broadcast on-chip
            wext = singles.tile([V, P * NCOL], F8)
            nc.scalar.dma_start(out=wext[:, : 32 * NCOL], in_=wext_in[:, : 32 * NCOL])
            nc.scalar.dma_start(out=wext[:, 32 * NCOL :], in_=wext_in[:, 32 * NCOL :])
            # host-broadcast w2e rides the otherwise-idle gpsimd ring, in
            # parallel with the first x tile; first scan needs only piece 1
            w2e = singles.tile([128, P * NCOL], BF16)
            nc.gpsimd.dma_start(out=w2e[:, : 32 * NCOL], in_=w2e_in[:, : 32 * NCOL])
            nc.gpsimd.dma_start(out=w2e[:, 32 * NCOL :], in_=w2e_in[:, 32 * NCOL :])

            for bt in range(NBT):
                # one batch tile: xt[v, p*128+b] for all 128 pathways, fp8.
                # 4 chunks of 32 pathways spread over all three DMA rings
                # (two rings measured only ~200 GB/s aggregate).
                xt = xtp.tile([128, P * 128], F8, tag="xt")
                for ch, eng in enumerate((nc.sync, nc.scalar, nc.gpsimd, nc.sync)):
                    eng.dma_start(
                        out=xt[:, ch * 32 * 128 : (ch + 1) * 32 * 128],
                        in_=xt_in[:, (bt * P + ch * 32) * 128 : (bt * P + (ch + 1) * 32) * 128],
                    )
                p_sb = psbp.tile([128, P], F32)
                for half in range(2):
                    # one shared prefix-sum scratch per half: 5 blocks of 15
                    # segments (66 cols each; G8's tail unused), each block led
                    # by a dummy segment whose end col is zeroed so per-pathway
                    # sums are boundary differences batched across blocks
                    prod = prodp.tile([128, 5 * 15 * NCOL], F32)
                    pr4 = prod[:].rearrange("p (g j c) -> p g j c", g=5, c=NCOL)
                    nc.scalar.memzero(pr4[:, :, 0:1, NCOL - 1 : NCOL])
                    for gi, (gs, G) in enumerate(GROUPS):
                        g2 = G // 2
                        h_ps = hpsp.tile([128, 1024], F32)
                        for j in range(G):
                            pa = half * 64 + gs + j
                            off = (j // g2) * 512 + (j % g2) * NCOL
                            nc.tensor.matmul(
                                h_ps[:, off : off + NCOL],
                                lhsT=xt[:, pa * 128 : (pa + 1) * 128],
                                rhs=wext[:, pa * NCOL : (pa + 1) * NCOL],
                                start=True,
                                stop=True,
                            )
                        h3d = h_ps[:].rearrange("p (b c) -> p b c", b=2)[
                            :, :, : g2 * NCOL
                        ]
                        w2s = w2e[
                            :, (half * 64 + gs) * NCOL : (half * 64 + gs + G) * NCOL
                        ]
                        # prod[t] = prefix-sum of w2e * relu(h) over the group
                        blk = prod[
                            :, gi * 15 * NCOL + NCOL : gi * 15 * NCOL + (G + 1) * NCOL
                        ]
                        nc.vector._custom_dve(
                            PREFIX_SUM_OP,
                            out=blk.rearrange("p (b c) -> p b c", b=2),
                            in0=w2s.rearrange("p (b c) -> p b c", b=2),
                            in1=h3d,
                        )
                    # per-pathway sums = differences of segment-end columns,
                    # batched: one sub for the four G14 blocks, one for G8
                    base = half * 64
                    ends = pr4[:, :, :, NCOL - 1 : NCOL]
                    nc.vector.tensor_sub(
                        out=p_sb[:, base : base + 56].rearrange(
                            "p (g j) -> p g j", g=4
                        ),
                        in0=ends[:, 0:4, 1:15, :].rearrange("p g j c -> p g (j c)"),
                        in1=ends[:, 0:4, 0:14, :].rearrange("p g j c -> p g (j c)"),
                    )
                    nc.vector.tensor_sub(
                        out=p_sb[:, base + 56 : base + 64],
                        in0=ends[:, 4:5, 1:9, :].rearrange("p g j c -> p (g j c)"),
                        in1=ends[:, 4:5, 0:8, :].rearrange("p g j c -> p (g j c)"),
                    )
                # whole-tile tail: final leaky on the idle ScalarE + one store
                pf = pfp.tile([128, P], F32)
                nc.scalar.activation(
                    out=pf[:],
                    in_=p_sb[:],
                    func=mybir.ActivationFunctionType.Lrelu,
                    alpha=0.2,
                )
                # output rides the scalar HWDGE ring (ACT is idle; the old
                # GpSimd SWDGE trigger cost 644ns per descriptor)
                nc.scalar.dma_start(
                    out=p_out[bt * 128 : (bt + 1) * 128, :],
                    in_=pf[:],
                )
    nc.finalize()
    return nc


def _prep_weights(W1, W2):
    W1T = np.ascontiguousarray(np.transpose(W1, (0, 2, 1)))          # [P,V,W]
    u = 0.2 * np.einsum("pvw,pw->pv", W1T, W2).astype(np.float32)    # [P,V]
    wext = np.concatenate(
        [S_W1 * W1T, S_U * u[:, :, None], -S_U * u[:, :, None]], axis=2
    )                                                                 # [P,V,66]
    wext = np.ascontiguousarray(np.transpose(wext, (1, 0, 2))).reshape(V, P * NCOL)
    wext_f8 = wext.astype(ml_dtypes.float8_e4m3)
    w2e = np.concatenate(
        [
            (0.8 / S_W1) * W2.astype(np.float32),
            (1.0 / S_U) * np.ones((P, 1), np.float32),
            (-1.0 / S_U) * np.ones((P, 1), np.float32),
        ],
        axis=1,
    ).reshape(1, P * NCOL).astype(ml_dtypes.bfloat16)                 # [1, P*66]
    w2ext = np.ascontiguousarray(np.broadcast_to(w2e, (128, P * NCOL)))
    return wext_f8, w2ext


def _prep_x(x):
    """fp8 cast + transpose to xt[core][v, (bt, p, b)]."""
    x_f8 = x.astype(ml_dtypes.float8_e4m3)
    xt = x_f8.reshape(N_CORES, NBT, 128, P, V).transpose(0, 4, 1, 3, 2)
    return np.ascontiguousarray(xt).reshape(N_CORES, V, NBT * P * 128)


def kernel(x, W1, W2, gamma, beta, Wd, bd):
    global LAST_RESULTS
    x = np.asarray(x, dtype=np.float32)
    W1 = np.asarray(W1, dtype=np.float32)
    W2 = np.asarray(W2, dtype=np.float32)

    if "nc" not in _CACHE:
        _CACHE["nc"] = _build_program()
    nc = _CACHE["nc"]

    wext_f8, w2e = _prep_weights(W1, W2)
    xt = _prep_x(x)
    in_maps = [
        {
            "xt": xt[c],
            "wext": wext_f8,
            "w2e": w2e,
        }
        for c in range(N_CORES)
    ]
    res = run_bass_kernel_spmd(nc, in_maps, list(range(N_CORES)))
    LAST_RESULTS = res

    pvals = np.concatenate(
        [res.results[c]["ps"] for c in range(N_CORES)], axis=0
    ).astype(np.float64)                                              # [B, P]

    mean = pvals.mean(axis=0)
    var = pvals.var(axis=0)
    pn = (pvals - mean) / np.sqrt(var + BN_EPS) * np.asarray(gamma, np.float64) \
        + np.asarray(beta, np.float64)
    pn = pn / np.linalg.norm(pn)
    out = 1.0 / (1.0 + np.exp(-(pn @ np.asarray(Wd, np.float64)
                                + np.asarray(bd, np.float64))))
    return out.astype(np.float32)


# revision 25
# speedup vs baseline: 1.0007x; 1.0007x over previous
"""DeepHisCoM Trainium2 kernel (nn_DeepHisCoM_7017976562218).

Math (reference):
    xr = x.reshape(B, P, V)
    z1 = einsum('bpv,pwv->bpw', xr, W1);  h = leaky(z1)          # per-pathway Linear V->W
    z2 = einsum('bpw,pw->bp', h, W2);     pval = leaky(z2)       # per-pathway Linear W->1
    BN(batch stats) -> global L2 normalize -> sigmoid(pn @ Wd + bd)

Device strategy (8 NeuronCores, batch-sharded 2048 rows/core):
    - x is cast to fp8(e4m3) and pre-transposed on the HOST into
      xt[v, (bt, p, b)] so the contraction dim V lands on partitions with
      zero device-side transposes (the old kernel burned half the PE on
      128x128 transposes, which also kept the HAM clock gate cold).
    - One 66-column fp8 matmul per pathway: lhsT = xt block (stationary,
      128 cols -> FWL fp8 weight load), rhs = [16*W1p^T | +256u | -256u]
      where u = 0.2 * W1p^T @ W2p.  leaky(z1) = 0.2*z1 + 0.8*relu(z1), so
      z2 = sum_w relu(z1)*0.8*W2 + (relu(q) - relu(-q)) with q = 0.2*sum_w z1*W2
      carried exactly by the +/-u columns through the uniform relu.
      Power-of-2 column scales (16, 256) keep fp8 operands in the normal
      range; the exact inverse rides in the bf16 w2e coefficients.
    - VectorE: fused prefix-sum of (w2e * relu(h)) in one custom DVE op,
      per-pathway sums recovered as boundary-column differences; final
      leaky via max(0.2*z, z).
    - BN stats + L2 norm + final linear + sigmoid on host (8 MiB, trivial).

fp8 is safe here: the global L2 norm + sigmoid make the logits tiny;
full-pipeline rel err vs the fp32 reference measured ~8e-6.
"""

import os
import sys

import numpy as np

for _p in ("/opt/trn_rl_repo",):
    if _p not in sys.path and os.path.isdir(_p):
        sys.path.insert(0, _p)

import ml_dtypes

import concourse.bacc as bacc
import concourse.bass as bass
import concourse.mybir as mybir
from concourse import dve_ops
from concourse.bass_utils import run_bass_kernel_spmd
from concourse.dve_spec import AluOp, Spec, Src0, Src1, Zero, relu, scan
from concourse.tile import TileContext


def _register_prefix_sum_op():
    """Fused DVE op: out[t] = running sum of in0[t] * relu(in1[t]).

    Per-pathway sums are recovered afterwards as differences of the
    segment-boundary columns of the prefix sum.
    """
    name = "STT_PREFIX_SUM_ANT"
    for op in dve_ops.OPS:
        if op.name == name:
            return op

    def ref(in0, in1, s0, s1, imm2):
        return np.cumsum(in0.astype(np.float32) * np.maximum(in1, 0), axis=-1)

    op = dve_ops.DveOp(
        name,
        Spec(body=scan(AluOp.ADD, Src0 * relu(Src1), init=Zero), reference=ref),
        subdim=False,
        uops_sha={"v3": "0179e875ac56dbc9", "v4": "d52b99774727e4db"},
    )
    dve_ops.OPS.append(op)
    dve_ops._SUB_OPCODE_FOR_NAME[name] = dve_ops._CUSTOM_DVE_ROW_BASE + len(dve_ops.OPS) - 1
    dve_ops.CUSTOM_DVE_SPECS[name] = op.spec
    return op


PREFIX_SUM_OP = _register_prefix_sum_op()

P, V, W = 128, 128, 64
B = 16384
N_CORES = 8
BSH = B // N_CORES          # 2048 batch rows per core
NBT = BSH // 128            # 16 batch tiles per core
BN_EPS = 1e-5
NCOL = W + 2                # 66: W1^T columns + (+u, -u)
S_W1 = 16.0                 # fp8 scale for the W1^T columns
S_U = 256.0                 # fp8 scale for the +/-u columns
F32 = mybir.dt.float32
BF16 = mybir.dt.bfloat16
F8 = mybir.dt.float8e4

# pathway groups per 64-pathway half: (start, size); size split across 2 PSUM banks
GROUPS = [(0, 14), (14, 14), (28, 14), (42, 14), (56, 8)]

_CACHE = {}
LAST_RESULTS = None


def _build_program():
    nc = bacc.Bacc()
    xt_in = nc.declare_dram_parameter("xt", [V, NBT * P * 128], F8, isOutput=False)
    wext_in = nc.declare_dram_parameter("wext", [V, P * NCOL], F8, isOutput=False)
    w2e_in = nc.declare_dram_parameter("w2e", [1, P * NCOL], BF16, isOutput=False)
    p_out = nc.declare_dram_parameter("ps", [BSH, P], F32, isOutput=True)

    with TileContext(nc) as tc:
        with (
            tc.tile_pool(name="singles", bufs=1) as singles,
            tc.tile_pool(name="xt", bufs=3) as xtp,
            tc.tile_pool(name="prod", bufs=2) as prodp,
            tc.tile_pool(name="psb", bufs=2) as psbp,
            tc.tile_pool(name="pf", bufs=2) as pfp,
            tc.tile_pool(name="hps", bufs=4, space="PSUM") as hpsp,
        ):
            # weight loads: wext rides the scalar ring in two halves with the
            # first x chunk between them (per-ring FIFO at ~80 GB/s would
            # otherwise stall the chunk); w2e loads as one 16 KiB row and is
            # partition-broadcast on the idle GpSimd, also split in halves
            wext = singles.tile([V, P * NCOL], F8)
            nc.scalar.dma_start(out=wext[:, : 64 * NCOL], in_=wext_in[:, : 64 * NCOL])
            w2e_row = singles.tile([1, P * NCOL], BF16)
            nc.gpsimd.dma_start(out=w2e_row[:], in_=w2e_in[:, :])
            w2e = singles.tile([128, P * NCOL], BF16)
            nc.gpsimd.partition_broadcast(
                w2e[:, : 64 * NCOL], w2e_row[:, : 64 * NCOL]
            )

            for bt in range(NBT):
                # one batch tile: xt[v, p*128+b] for all 128 pathways, fp8.
                # 4 chunks of 32 pathways spread over all three DMA rings
                # (two rings measured only ~200 GB/s aggregate).
                xt = xtp.tile([128, P * 128], F8, tag="xt")
                for ch, eng in enumerate((nc.sync, nc.scalar, nc.gpsimd, nc.sync)):
                    eng.dma_start(
                        out=xt[:, ch * 32 * 128 : (ch + 1) * 32 * 128],
                        in_=xt_in[:, (bt * P + ch * 32) * 128 : (bt * P + (ch + 1) * 32) * 128],
                    )
                if bt == 0:
                    # second halves of the weights, behind tile 0's chunks in
                    # their rings' FIFOs but ahead of tile 1's
                    nc.scalar.dma_start(
                        out=wext[:, 64 * NCOL :], in_=wext_in[:, 64 * NCOL :]
                    )
                    nc.gpsimd.partition_broadcast(
                        w2e[:, 64 * NCOL :], w2e_row[:, 64 * NCOL :]
                    )
                p_sb = psbp.tile([128, P], F32)
                for half in range(2):
                    # one shared prefix-sum scratch per half: 5 blocks of 15
                    # segments (66 cols each; G8's tail unused), each block led
                    # by a dummy segment whose end col is zeroed so per-pathway
                    # sums are boundary differences batched across blocks
                    prod = prodp.tile([128, 5 * 15 * NCOL], F32)
                    pr4 = prod[:].rearrange("p (g j c) -> p g j c", g=5, c=NCOL)
                    nc.scalar.memzero(pr4[:, :, 0:1, NCOL - 1 : NCOL])
                    for gi, (gs, G) in enumerate(GROUPS):
                        g2 = G // 2
                        h_ps = hpsp.tile([128, 1024], F32)
                        for j in range(G):
                            pa = half * 64 + gs + j
                            off = (j // g2) * 512 + (j % g2) * NCOL
                            nc.tensor.matmul(
                                h_ps[:, off : off + NCOL],
                                lhsT=xt[:, pa * 128 : (pa + 1) * 128],
                                rhs=wext[:, pa * NCOL : (pa + 1) * NCOL],
                                start=True,
                                stop=True,
                            )
                        h3d = h_ps[:].rearrange("p (b c) -> p b c", b=2)[
                            :, :, : g2 * NCOL
                        ]
                        w2s = w2e[
                            :, (half * 64 + gs) * NCOL : (half * 64 + gs + G) * NCOL
                        ]
                        # prod[t] = prefix-sum of w2e * relu(h) over the group
                        blk = prod[
                            :, gi * 15 * NCOL + NCOL : gi * 15 * NCOL + (G + 1) * NCOL
                        ]
                        nc.vector._custom_dve(
                            PREFIX_SUM_OP,
                            out=blk.rearrange("p (b c) -> p b c", b=2),
                            in0=w2s.rearrange("p (b c) -> p b c", b=2),
                            in1=h3d,
                        )
                    # per-pathway sums = differences of segment-end columns,
                    # batched: one sub for the four G14 blocks, one for G8
                    base = half * 64
                    ends = pr4[:, :, :, NCOL - 1 : NCOL]
                    nc.vector.tensor_sub(
                        out=p_sb[:, base : base + 56].rearrange(
                            "p (g j) -> p g j", g=4
                        ),
                        in0=ends[:, 0:4, 1:15, :].rearrange("p g j c -> p g (j c)"),
                        in1=ends[:, 0:4, 0:14, :].rearrange("p g j c -> p g (j c)"),
                    )
                    nc.vector.tensor_sub(
                        out=p_sb[:, base + 56 : base + 64],
                        in0=ends[:, 4:5, 1:9, :].rearrange("p g j c -> p (g j c)"),
                        in1=ends[:, 4:5, 0:8, :].rearrange("p g j c -> p (g j c)"),
                    )
                # whole-tile tail: final leaky on the idle ScalarE + one store
                pf = pfp.tile([128, P], F32)
                nc.scalar.activation(
                    out=pf[:],
                    in_=p_sb[:],
                    func=mybir.ActivationFunctionType.Lrelu,
                    alpha=0.2,
                )
                # output rides the scalar HWDGE ring (ACT is idle; the old
                # GpSimd SWDGE trigger cost 644ns per descriptor)
                nc.scalar.dma_start(
                    out=p_out[bt * 128 : (bt + 1) * 128, :],
                    in_=pf[:],
                )
    nc.finalize()
    return nc


def _prep_weights(W1, W2):
    W1T = np.ascontiguousarray(np.transpose(W1, (0, 2, 1)))          # [P,V,W]
    u = 0.2 * np.einsum("pvw,pw->pv", W1T, W2).astype(np.float32)    # [P,V]
    wext = np.concatenate(
        [S_W1 * W1T, S_U * u[:, :, None], -S_U * u[:, :, None]], axis=2
    )                                                                 # [P,V,66]
    wext = np.ascontiguousarray(np.transpose(wext, (1, 0, 2))).reshape(V, P * NCOL)
    wext_f8 = wext.astype(ml_dtypes.float8_e4m3)
    w2e = np.concatenate(
        [
            (0.8 / S_W1) * W2.astype(np.float32),
            (1.0 / S_U) * np.ones((P, 1), np.float32),
            (-1.0 / S_U) * np.ones((P, 1), np.float32),
        ],
        axis=1,
    ).reshape(1, P * NCOL).astype(ml_dtypes.bfloat16)                 # [1, P*66]
    return wext_f8, np.ascontiguousarray(w2e)


def _prep_x(x):
    """fp8 cast + transpose to xt[core][v, (bt, p, b)]."""
    x_f8 = x.astype(ml_dtypes.float8_e4m3)
    xt = x_f8.reshape(N_CORES, NBT, 128, P, V).transpose(0, 4, 1, 3, 2)
    return np.ascontiguousarray(xt).reshape(N_CORES, V, NBT * P * 128)


def kernel(x, W1, W2, gamma, beta, Wd, bd):
    global LAST_RESULTS
    x = np.asarray(x, dtype=np.float32)
    W1 = np.asarray(W1, dtype=np.float32)
    W2 = np.asarray(W2, dtype=np.float32)

    if "nc" not in _CACHE:
        _CACHE["nc"] = _build_program()
    nc = _CACHE["nc"]

    wext_f8, w2e = _prep_weights(W1, W2)
    xt = _prep_x(x)
    in_maps = [
        {
            "xt": xt[c],
            "wext": wext_f8,
            "w2e": w2e,
        }
        for c in range(N_CORES)
    ]
    res = run_bass_kernel_spmd(nc, in_maps, list(range(N_CORES)))
    LAST_RESULTS = res

    pvals = np.concatenate(
        [res.results[c]["ps"] for c in range(N_CORES)], axis=0
    ).astype(np.float64)                                              # [B, P]

    mean = pvals.mean(axis=0)
    var = pvals.var(axis=0)
    pn = (pvals - mean) / np.sqrt(var + BN_EPS) * np.asarray(gamma, np.float64) \
        + np.asarray(beta, np.float64)
    pn = pn / np.linalg.norm(pn)
    out = 1.0 / (1.0 + np.exp(-(pn @ np.asarray(Wd, np.float64)
                                + np.asarray(bd, np.float64))))
    return out.astype(np.float32)
